# revision 3
# baseline (speedup 1.0000x reference)
"""2-layer GCN (PyG GCNConv semantics) on 8 Trainium2 NeuronCores — v2.

Strategy (vertex-cut, dst-partitioned edges):
 - nodes split contiguously across 8 cores (12500 each, padded to 12544)
 - per-core transform: g = (x @ W) * dis in feature tiles; compact [SLP, 64]
   f16 slice AllGathered to a full table viewed as pair rows [V/2, 256B]
   (one gather row carries TWO nodes; 256B is the dma_gather minimum)
 - aggregation: edges grouped by (dst-window 64, table chunk of 32768 pair
   rows, src parity); host-built slot streams; dma_gather over 4 SWDGE
   queues pulls 1024-slot blocks; per-block PE matmul with a one-hot
   selection matrix S (S[slot, dst] = dis[dst]) reduces onto dst windows
 - self-loop term is applied locally during the transform (acc := ps*dis^2),
   so self edges never enter the gather streams
 - per-window PSUM partials accumulate into SBUF; bias+ReLU -> layer-2
   transform -> AllGather -> same aggregation -> +bias -> output
"""
import math
import os

import numpy as np

P = 128
D = 64
NCORES = 8
CW = 32768          # int16-addressable chunk window (pair rows)
B_SLOTS = int(os.environ.get("GCN_B", "1024"))  # slots per dma_gather call (= ring capacity)
WIN = 64            # dsts per S-matmul window
NQ = 4              # SWDGE queues for gather parallelism
IDX_STRIP = 16384 // B_SLOTS   # gather calls per idx strip load (16k slots)
S_STRIP = 4         # windows per S strip load
SINGLE_PACKET = os.environ.get("GCN_SP", "0") == "1"
GAT_BUFS = int(os.environ.get("GCN_GAT_BUFS", "8"))
EAGER = os.environ.get("GCN_EAGER", "0") == "1"


def _host_prep(x, edge_index, W1, b1, W2, b2, SL):
    """Build all per-core device inputs. SL = nodes per core."""
    N = x.shape[0]
    assert N == NCORES * SL
    SLP = ((SL + P - 1) // P) * P          # padded slice rows
    NT = SLP // P                           # 128-dst tiles per core
    NW = SLP // WIN                         # 64-dst windows per core
    V = NCORES * SLP                        # padded node count
    V2 = V // 2                             # pair rows in the table
    n_chunks = (V2 + CW - 1) // CW

    src = np.asarray(edge_index[0], dtype=np.int64)
    dst = np.asarray(edge_index[1], dtype=np.int64)

    deg = np.bincount(dst, minlength=N).astype(np.float64) + 1.0
    dis = (1.0 / np.sqrt(deg)).astype(np.float32)

    gp_ = (src // SL) * SLP + (src % SL)    # padded global src id
    pr_all = gp_ >> 1                       # pair row
    hf_all = (gp_ & 1).astype(np.int64)
    core_of = dst // SL

    # --- per-core sorted edge structures ---
    per_core = []
    NG = NW * n_chunks * 2                  # (w, k, h) group count
    cnt = np.zeros((NCORES, NG), dtype=np.int64)
    for c in range(NCORES):
        m = core_of == c
        ld = (dst[m] - c * SL).astype(np.int64)
        pr = pr_all[m]
        hf = hf_all[m]
        dval = dis[dst[m]]
        w = ld // WIN
        kk = (pr // CW).astype(np.int64)
        gid = (w * n_chunks + kk) * 2 + hf
        order = np.lexsort((pr, gid))
        ld, pr, hf, dval, gid = (a[order] for a in (ld, pr, hf, dval, gid))
        per_core.append((ld, pr, dval, gid))
        np.add.at(cnt[c], gid, 1)

    cnt_max = cnt.max(axis=0)               # [NG]
    nb = (cnt_max + P - 1) // P             # blocks per (w,k,h) group

    # group order in chunk-k stream: w-major, then h. S columns follow
    # the matmul order: k-major, then w, then h.
    nb_wkh = nb.reshape(NW, n_chunks, 2)
    # slot base of group (w,h) within chunk k
    base_in_chunk = np.zeros((NW, n_chunks, 2), dtype=np.int64)
    gb_base = np.zeros((NW, n_chunks, 2), dtype=np.int64)
    L_chunk = np.zeros(n_chunks, dtype=np.int64)
    gb = 0
    for k in range(n_chunks):
        pos = 0
        for w in range(NW):
            for h in range(2):
                base_in_chunk[w, k, h] = pos
                pos += nb_wkh[w, k, h] * P
                gb_base[w, k, h] = gb
                gb += nb_wkh[w, k, h]
        L_chunk[k] = pos
    tot_blocks = int(gb)
    L_pad = [((int(L) + B_SLOTS - 1) // B_SLOTS) * B_SLOTS for k, L in
             enumerate(L_chunk)]

    # --- per-core S + idx streams (vectorized) ---
    S_dev = np.zeros((NCORES, P, tot_blocks * WIN), dtype=np.float16)
    idx_dev = [np.zeros((NCORES, P, L_pad[k] // 16), dtype=np.int16)
               for k in range(n_chunks)]
    grp_starts_tmpl = np.arange(NG)
    for c in range(NCORES):
        ld, pr, dval, gid = per_core[c]
        starts = np.searchsorted(gid, grp_starts_tmpl)
        r = np.arange(gid.shape[0]) - starts[gid]
        w = gid // (n_chunks * 2)
        kk = (gid // 2) % n_chunks
        hf = gid % 2
        slot = base_in_chunk[w, kk, hf] + r
        gbi = gb_base[w, kk, hf] + r // P
        sip = r % P
        scol = ld - w * WIN
        S_dev[c, sip, gbi * WIN + scol] = dval.astype(np.float16)
        for k in range(n_chunks):
            m = kk == k
            s = np.zeros(L_pad[k], dtype=np.int16)
            s[slot[m]] = (pr[m] - k * CW).astype(np.int16)
            wrp = s.reshape(-1, 16).T               # [16, L/16]
            idx_dev[k][c] = np.tile(wrp, (8, 1)).astype(np.int16)

    # schedule for codegen: per chunk, ordered (w, [(h, nblocks)...])
    sched = []
    for k in range(n_chunks):
        rows = []
        for w in range(NW):
            hb = [(h, int(nb_wkh[w, k, h])) for h in range(2)
                  if nb_wkh[w, k, h] > 0]
            if hb:
                rows.append((w, hb))
        sched.append(rows)

    # transform inputs
    xT = np.zeros((NCORES, D, SLP), dtype=np.float32)
    dis_sb = np.zeros((NCORES, P, NT), dtype=np.float32)
    dis2_sb = np.zeros((NCORES, P, NT), dtype=np.float32)
    for c in range(NCORES):
        xs = np.asarray(x[c * SL : (c + 1) * SL], dtype=np.float32)
        xT[c, :, :SL] = xs.T
        dp = np.zeros(SLP, dtype=np.float32)
        dp[:SL] = dis[c * SL : (c + 1) * SL]
        dis_sb[c] = dp.reshape(NT, P).T
        dis2_sb[c] = (dp * dp).reshape(NT, P).T

    b1b = np.tile(np.asarray(b1, np.float32)[None, :], (P, 1))
    b2b = np.tile(np.asarray(b2, np.float32)[None, :], (P, 1))

    meta = dict(
        SL=SL, SLP=SLP, NT=NT, NW=NW, V=V, V2=V2, n_chunks=n_chunks,
        L_chunk=[int(v) for v in L_chunk], L_pad=L_pad,
        tot_blocks=tot_blocks, sched=sched,
        gb_base=gb_base, base_in_chunk=base_in_chunk,
    )
    inputs = dict(
        xT=xT, dis_sb=dis_sb, dis2_sb=dis2_sb, S=S_dev, idx=idx_dev,
        b1b=b1b, b2b=b2b,
        W1=np.asarray(W1, np.float32), W2=np.asarray(W2, np.float32),
    )
    return meta, inputs


def _build_kernel(meta, timing_trips=0):
    import concourse.bass as bass
    import concourse.bacc as bacc
    import concourse.mybir as mybir
    import concourse.tile as tile
    from concourse.masks import make_identity

    SLP, NT, NW, V2 = meta["SLP"], meta["NT"], meta["NW"], meta["V2"]
    n_chunks, sched = meta["n_chunks"], meta["sched"]
    L_chunk, L_pad = meta["L_chunk"], meta["L_pad"]
    tot_blocks = meta["tot_blocks"]
    gb_base, base_in_chunk = meta["gb_base"], meta["base_in_chunk"]
    f32, f16, i16, i32 = (mybir.dt.float32, mybir.dt.float16, mybir.dt.int16,
                          mybir.dt.int32)

    nc = bacc.Bacc("TRN2", target_bir_lowering=False, debug=False,
                   num_devices=NCORES, num_swdge_queues=NQ,
                   dynamic_dma_scratch_size=16 * B_SLOTS)

    xT_t = nc.dram_tensor("xT", [D, SLP], f32, kind="ExternalInput")
    dis_t = nc.dram_tensor("dis_sb", [P, NT], f32, kind="ExternalInput")
    dis2_t = nc.dram_tensor("dis2_sb", [P, NT], f32, kind="ExternalInput")
    S_t = nc.dram_tensor("S", [P, tot_blocks * WIN], f16, kind="ExternalInput")
    idx_ts = [
        nc.dram_tensor(f"idx{k}", [P, L_pad[k] // 16], i16, kind="ExternalInput")
        for k in range(n_chunks)
    ]
    W1_t = nc.dram_tensor("W1", [D, D], f32, kind="ExternalInput")
    W2_t = nc.dram_tensor("W2", [D, D], f32, kind="ExternalInput")
    b1b_t = nc.dram_tensor("b1b", [P, D], f32, kind="ExternalInput")
    b2b_t = nc.dram_tensor("b2b", [P, D], f32, kind="ExternalInput")
    out_t = nc.dram_tensor("out", [SLP, D], f32, kind="ExternalOutput")

    # widest S strip (columns) across all (chunk, 4-window strips)
    def strip_list(k):
        rows = sched[k]
        return [rows[i : i + S_STRIP] for i in range(0, len(rows), S_STRIP)]

    max_strip_w = 0
    for k in range(n_chunks):
        for st in strip_list(k):
            wcols = sum(nbl for _, hb in st for _, nbl in hb) * WIN
            max_strip_w = max(max_strip_w, wcols)

    with tile.TileContext(nc) as tc:
        with (
            tc.tile_pool(name="const", bufs=1) as cp,
            tc.tile_pool(name="io", bufs=3) as iop,
            tc.tile_pool(name="gat", bufs=GAT_BUFS) as gp,
            tc.tile_pool(name="idxp", bufs=2) as ip,
            tc.tile_pool(name="spool", bufs=2) as sp,
            tc.tile_pool(name="acc", bufs=1) as ap,
            tc.tile_pool(name="psum", bufs=4, space="PSUM") as pp,
            tc.tile_pool(name="tps", bufs=2, space="PSUM") as tpp,
            tc.tile_pool(name="dram", bufs=1, space="DRAM") as dp,
        ):
            # ---- constants ----
            W1_sb = cp.tile([D, D], f32)
            W2_sb = cp.tile([D, D], f32)
            b1_sb = cp.tile([P, D], f32)
            b2_sb = cp.tile([P, D], f32)
            dis_sb = cp.tile([P, NT], f32)
            dis2_sb = cp.tile([P, NT], f32)
            xT_sb = cp.tile([D, SLP], f32)
            ident = cp.tile([P, P], f32)
            nc.sync.dma_start(out=W1_sb[:], in_=W1_t[:])
            nc.sync.dma_start(out=W2_sb[:], in_=W2_t[:])
            nc.sync.dma_start(out=b1_sb[:], in_=b1b_t[:])
            nc.sync.dma_start(out=b2_sb[:], in_=b2b_t[:])
            nc.sync.dma_start(out=dis_sb[:], in_=dis_t[:])
            nc.sync.dma_start(out=dis2_sb[:], in_=dis2_t[:])
            nc.sync.dma_start(out=xT_sb[:], in_=xT_t[:])
            make_identity(nc, ident[:])

            # DRAM bounce buffers (collectives need internal tiles)
            g_slice = dp.tile([SLP, D], f16)
            g1_full = dp.tile([V2, P], f16, addr_space="Shared")
            g2_full = dp.tile([V2, P], f16, addr_space="Shared")

            # accumulators
            h1pre = ap.tile([P, NT * D], f32)
            h2pre = ap.tile([P, NT * D], f32)

            # dummy indirect dma so walrus configures the pool-dynamic ring
            idx32_sb = cp.tile([P, 1], i32)
            dummy_sb = cp.tile([P, D], f32)
            nc.vector.memset(idx32_sb[:], 0)
            nc.gpsimd.indirect_dma_start(
                out=dummy_sb[:], out_offset=None, in_=b1b_t[:],
                in_offset=bass.IndirectOffsetOnAxis(ap=idx32_sb[:], axis=0),
            )

            qctr = [0]

            def transform(src_kind, j, acc):
                """node tile j: matmul -> g f16 row tile + local self-loop init"""
                if src_kind == 1:
                    lhsT = xT_sb[:, j * P : (j + 1) * P]
                    W_sb = W1_sb
                else:
                    t0 = iop.tile([P, D], f32, tag="t0")
                    nc.vector.tensor_tensor(
                        out=t0[:], in0=h1pre[:, j * D : (j + 1) * D],
                        in1=b1_sb[:], op=mybir.AluOpType.add,
                    )
                    h1 = iop.tile([P, D], f32, tag="h1")
                    nc.scalar.activation(
                        out=h1[:], in_=t0[:],
                        func=mybir.ActivationFunctionType.Relu,
                    )
                    tps = tpp.tile([D, P], f32, tag="tps")
                    nc.tensor.transpose(out=tps[:], in_=h1[:], identity=ident[:])
                    h1T = iop.tile([D, P], f32, tag="h1T")
                    nc.vector.tensor_copy(out=h1T[:], in_=tps[:])
                    lhsT = h1T[:]
                    W_sb = W2_sb
                ps = tpp.tile([P, D], f32, tag="tmm")
                nc.tensor.matmul(out=ps[:], lhsT=lhsT, rhs=W_sb[:],
                                 start=True, stop=True)
                gt = iop.tile([P, D], f16, tag="gt")
                nc.vector.tensor_scalar(
                    out=gt[:], in0=ps[:], scalar1=dis_sb[:, j : j + 1],
                    scalar2=None, op0=mybir.AluOpType.mult,
                )
                # self-loop contribution initializes the accumulator
                nc.vector.tensor_scalar(
                    out=acc[:, j * D : (j + 1) * D], in0=ps[:],
                    scalar1=dis2_sb[:, j : j + 1],
                    scalar2=None, op0=mybir.AluOpType.mult,
                )
                nc.sync.dma_start(out=g_slice[j * P : (j + 1) * P, :], in_=gt[:])

            def allgather(dst_full):
                if NCORES == 1:
                    nc.sync.dma_start(
                        out=dst_full[:].rearrange("a b -> (a b)"),
                        in_=g_slice[:].rearrange("a b -> (a b)"),
                    )
                else:
                    nc.gpsimd.collective_compute(
                        "AllGather", mybir.AluOpType.bypass,
                        replica_groups=[list(range(NCORES))],
                        ins=[g_slice.opt()], outs=[dst_full.opt()],
                    )

            def aggregate(table_full, acc):
                for k in range(n_chunks):
                    L = L_pad[k]
                    ncalls = (L + B_SLOTS - 1) // B_SLOTS
                    tab = table_full[k * CW : min((k + 1) * CW, V2), :]
                    call_tiles = [None] * ncalls
                    idx_tiles = {}

                    def ensure_call(j):
                        if call_tiles[j] is not None:
                            return
                        si = j // IDX_STRIP
                        if si not in idx_tiles:
                            it = ip.tile([P, IDX_STRIP * B_SLOTS // 16], i16,
                                         tag="idx")
                            o = si * IDX_STRIP * B_SLOTS // 16
                            n = min(IDX_STRIP * B_SLOTS // 16,
                                    L // 16 - o)
                            nc.sync.dma_start(
                                out=it[:, :n], in_=idx_ts[k][:, o : o + n],
                            )
                            idx_tiles[si] = it
                        it = idx_tiles[si]
                        co = (j % IDX_STRIP) * (B_SLOTS // 16)
                        gt = gp.tile([P, B_SLOTS // P, P], f16, tag="g")
                        nc.gpsimd.dma_gather(
                            gt[:], tab, it[:, co : co + B_SLOTS // 16],
                            B_SLOTS, B_SLOTS, P,
                            single_packet=SINGLE_PACKET,
                            queue_num=qctr[0] % NQ,
                        )
                        qctr[0] += 1
                        call_tiles[j] = gt

                    if EAGER:
                        for j in range(ncalls):
                            ensure_call(j)

                    for strip in strip_list(k):
                        # S strip load
                        w0 = strip[0][0]
                        h0 = strip[0][1][0][0]
                        bi0 = int(gb_base[w0, k, h0])
                        wcols = sum(nbl for _, hb in strip
                                    for _, nbl in hb) * WIN
                        St = sp.tile([P, max_strip_w], f16, tag="S")
                        nc.sync.dma_start(
                            out=St[:, :wcols],
                            in_=S_t[:, bi0 * WIN : bi0 * WIN + wcols],
                        )
                        for w, hb in strip:
                            ps = pp.tile([P, D], f32, tag="ps")
                            half = (w % 2) * WIN
                            out_ps = ps[half : half + WIN, :]
                            nblocks = sum(nbl for _, nbl in hb)
                            bdone = 0
                            for h, nbl in hb:
                                gbase = int(gb_base[w, k, h])
                                sbase = int(base_in_chunk[w, k, h])
                                for b in range(nbl):
                                    slot = sbase + b * P
                                    j = slot // B_SLOTS
                                    ensure_call(j)
                                    ct = call_tiles[j]
                                    s_in = (slot % B_SLOTS) // P
                                    scol = (gbase + b - bi0) * WIN
                                    nc.tensor.matmul(
                                        out=out_ps,
                                        lhsT=St[:, scol : scol + WIN],
                                        rhs=ct[:, s_in,
                                               h * D : (h + 1) * D],
                                        start=(bdone == 0),
                                        stop=(bdone == nblocks - 1),
                                    )
                                    bdone += 1
                            wcol = (w // 2) * D
                            nc.vector.tensor_tensor(
                                out=acc[half : half + WIN, wcol : wcol + D],
                                in0=acc[half : half + WIN, wcol : wcol + D],
                                in1=out_ps, op=mybir.AluOpType.add,
                            )

            # ---- pipeline ----
            def layer1():
                for j in range(NT):
                    transform(1, j, h1pre)

            def tail():
                for j in range(NT):
                    transform(2, j, h2pre)

            def outs():
                for j in range(NT):
                    ot = iop.tile([P, D], f32, tag="ot")
                    nc.vector.tensor_tensor(
                        out=ot[:], in0=h2pre[:, j * D : (j + 1) * D],
                        in1=b2_sb[:], op=mybir.AluOpType.add,
                    )
                    nc.sync.dma_start(out=out_t[j * P : (j + 1) * P, :],
                                      in_=ot[:])

            if timing_trips:
                layer1()
                allgather(g1_full)
                allgather(g2_full)
                with tc.For_i(0, timing_trips, 1):
                    aggregate(g1_full, h1pre)
                    tail()
                    aggregate(g2_full, h2pre)
                    outs()
            else:
                layer1()
                allgather(g1_full)
                aggregate(g1_full, h1pre)
                tail()
                allgather(g2_full)
                aggregate(g2_full, h2pre)
                outs()

    nc.compile()
    return nc


LAST_EXEC_NS = None
LAST_TRACE = None


def kernel(x, edge_index, W1, b1, W2, b2):
    import os

    import concourse.bass_utils as bass_utils

    x = np.asarray(x)
    N = x.shape[0]
    SL = N // NCORES
    meta, inp = _host_prep(x, edge_index, W1, b1, W2, b2, SL)
    nc = _build_kernel(meta)

    in_maps = []
    for c in range(NCORES):
        m = {
            "xT": inp["xT"][c], "dis_sb": inp["dis_sb"][c],
            "dis2_sb": inp["dis2_sb"][c], "S": inp["S"][c],
            "W1": inp["W1"], "W2": inp["W2"], "b1b": inp["b1b"],
            "b2b": inp["b2b"],
        }
        for k in range(meta["n_chunks"]):
            m[f"idx{k}"] = inp["idx"][k][c]
        in_maps.append(m)

    res = bass_utils.run_bass_kernel_spmd(
        nc, in_maps, core_ids=list(range(NCORES))
    )
    out = np.empty((N, D), dtype=np.float32)
    for c in range(NCORES):
        out[c * SL : (c + 1) * SL] = res.results[c]["out"][:SL]
    return out
